# revision 13
# baseline (speedup 1.0000x reference)
"""Trainium2 Bass kernel for nn_EncoderBlock (B=2, L=2048, D=1024, H=16, FF=4096).

Sharding: sequence-parallel over the 4096 tokens across 8 cores (512 tokens
per core; cores 0-3 own batch 0, cores 4-7 own batch 1). Per-core work is
dense (full weights). Collectives (all overlapped with compute):
  - AllGather (4-core groups) of the PRE-normalization Q projection (bf16),
    triggered first so the runtime's collective bootstrap barrier and the
    NEFF-launch skew burn during the K/V matmuls. The LN fixup is affine
    and per-partition with identical stats across the group, so it is
    applied to the gathered result instead.
  - AllReduce (8 cores) of masked LayerNorm partial sums (LN1, LN2), 16B
    each. QKV / FFN1 matmuls run on the RAW input concurrently; the
    normalization is applied afterwards as an affine fixup:
    W @ ((x - mu) * rs) = rs * (W @ x) - rs * mu * rowsum(W).
  - 8x ReduceScatter (4-core groups) of partial attention numerators and
    softmax denominators (additive across key shards); each core receives
    the full-key sums for its own 512 queries. Pipelined per 2-head group,
    with normalization interleaved one group behind the triggers.

Each core scores ALL 2048 queries of its batch against its LOCAL 512
keys/values. Activations keep the feature dim on partitions and tokens on
the free axis so matmuls contract along partitions with no transposes.
Weights are pre-transposed and cast to bf16 on the host. Softmax
denominators come from a ones-column appended to V (row 64 of the att@v
accumulator).

Dtypes: bf16 operands for all matmuls; fp32 stats/residuals/PSUM accum.
"""

import sys

sys.path.insert(0, "/opt/trn_rl_repo")

from contextlib import ExitStack  # noqa: E402

import numpy as np  # noqa: E402
import ml_dtypes  # noqa: E402

import concourse.bass as bass  # noqa: E402
import concourse.mybir as mybir  # noqa: E402
import concourse.tile as tile  # noqa: E402
from concourse import bacc, bass_utils  # noqa: E402

B, L, D, H, FF = 2, 2048, 1024, 16, 4096
DH = D // H  # 64
NCORES = 8
RANKS = 4  # cores per batch group
T = B * L // NCORES  # 512 tokens per core
KC = D // 128  # 8 contraction chunks of 128
HP = H // 2  # 8 head-pairs (2 heads per 128-partition chunk)
FM = FF // 128  # 32 ff chunks
NTOT = float(L * D)  # layernorm element count per batch
EPS = 1e-5
SCALE = 1.0 / np.sqrt(np.float32(H))  # faithful to source bug: 1/sqrt(H)

F32 = mybir.dt.float32
BF16 = mybir.dt.bfloat16

Q_ELEMS = 128 * HP * T  # qpre [128, 8, 512] bf16
HG = 2  # heads per ReduceScatter group
NRS = H // HG  # 8 ReduceScatter ops

_CACHE = {}


def _ap(t, offset, dims):
    """Manual AP over a dram tile: dims = [(step, count), ...], partition first."""
    return bass.AP(
        tensor=t.tensor, offset=t.offset + offset, ap=[[s, c] for s, c in dims]
    )


def _ln_stats_pre(nc, const, tiny, ps, src, msel_sb, ones, ar_in, pfx):
    """Partial LN sums of src -> masked [1,4] staged in ar_in (DRAM)."""
    AF = mybir.ActivationFunctionType
    s_part = tiny.tile([128, 1], F32, tag=pfx + "_s")
    nc.vector.tensor_reduce(
        out=s_part, in_=src, axis=mybir.AxisListType.XY, op=mybir.AluOpType.add
    )
    junk = const.tile([128, KC, T], BF16, tag="junk")
    q_part = tiny.tile([128, 1], F32, tag=pfx + "_q")
    nc.scalar.activation(out=junk, in_=src, func=AF.Square, accum_out=q_part)
    st2 = tiny.tile([128, 2], F32, tag=pfx + "_st2")
    nc.vector.tensor_copy(out=st2[:, 0:1], in_=s_part)
    nc.vector.tensor_copy(out=st2[:, 1:2], in_=q_part)
    ps_st = ps.tile([1, 2], F32, tag="ps")
    nc.tensor.matmul(ps_st, ones, st2, start=True, stop=True)
    sb4 = tiny.tile([1, 4], F32, tag=pfx + "_sb4")
    nc.scalar.copy(out=sb4[0:1, 0:2], in_=ps_st)
    nc.scalar.copy(out=sb4[0:1, 2:4], in_=ps_st)
    nc.vector.tensor_mul(out=sb4, in0=sb4, in1=msel_sb)
    nc.sync.dma_start(out=ar_in, in_=sb4)


def _ln_stats_post(nc, tiny, msel_sb, eps_t, ar_in, ar_out, rg_all, pfx):
    """AllReduce the staged sums; derive (mu_b, rs_b, nrsmu_b) [128,1]."""
    AF = mybir.ActivationFunctionType
    nc.gpsimd.collective_compute(
        "AllReduce", mybir.AluOpType.add, replica_groups=rg_all,
        ins=[ar_in.opt()], outs=[ar_out.opt()],
    )
    r4 = tiny.tile([1, 4], F32, tag=pfx + "_r4")
    nc.sync.dma_start(out=r4, in_=ar_out)
    nc.vector.tensor_mul(out=r4, in0=r4, in1=msel_sb)
    sq2 = tiny.tile([1, 2], F32, tag=pfx + "_sq2")
    nc.vector.tensor_tensor(
        out=sq2, in0=r4[0:1, 0:2], in1=r4[0:1, 2:4], op=mybir.AluOpType.add
    )
    mean = tiny.tile([1, 1], F32, tag=pfx + "_mean")
    nc.scalar.mul(out=mean, in_=sq2[0:1, 0:1], mul=1.0 / NTOT)
    e2 = tiny.tile([1, 1], F32, tag=pfx + "_e2")
    nc.scalar.mul(out=e2, in_=sq2[0:1, 1:2], mul=1.0 / NTOT)
    musq = tiny.tile([1, 1], F32, tag=pfx + "_musq")
    nc.vector.tensor_mul(out=musq, in0=mean, in1=mean)
    var = tiny.tile([1, 1], F32, tag=pfx + "_var")
    nc.vector.tensor_tensor(
        out=var, in0=e2, in1=musq, op=mybir.AluOpType.subtract
    )
    sd = tiny.tile([1, 1], F32, tag=pfx + "_sd")
    nc.scalar.activation(out=sd, in_=var, func=AF.Sqrt, bias=eps_t)
    rs = tiny.tile([1, 1], F32, tag=pfx + "_rs")
    nc.vector.reciprocal(out=rs, in_=sd)
    rsmu = tiny.tile([1, 1], F32, tag=pfx + "_rsmu")
    nc.vector.tensor_mul(out=rsmu, in0=mean, in1=rs)
    nrsmu = tiny.tile([1, 1], F32, tag=pfx + "_nrsmu")
    nc.scalar.mul(out=nrsmu, in_=rsmu, mul=-1.0)
    mu_b = tiny.tile([128, 1], F32, tag=pfx + "_mub")
    rs_b = tiny.tile([128, 1], F32, tag=pfx + "_rsb")
    nrsmu_b = tiny.tile([128, 1], F32, tag=pfx + "_nmb")
    nc.gpsimd.partition_broadcast(mu_b, mean)
    nc.gpsimd.partition_broadcast(rs_b, rs)
    nc.gpsimd.partition_broadcast(nrsmu_b, nrsmu)
    return mu_b, rs_b, nrsmu_b


def _build():
    nc = bacc.Bacc("TRN2", target_bir_lowering=False, debug=False,
                   num_devices=NCORES)

    x_t = nc.dram_tensor("x_t", [D, T], F32, kind="ExternalInput")
    wq_t = nc.dram_tensor("wq_t", [D, D], BF16, kind="ExternalInput")
    wk_t = nc.dram_tensor("wk_t", [D, D], BF16, kind="ExternalInput")
    wv_t = nc.dram_tensor("wv_t", [D, D], BF16, kind="ExternalInput")
    wo_t = nc.dram_tensor("wo_t", [D, D], BF16, kind="ExternalInput")
    w1_t = nc.dram_tensor("w1_t", [D, FF], BF16, kind="ExternalInput")
    w2_t = nc.dram_tensor("w2_t", [FF, D], BF16, kind="ExternalInput")
    bq_s = nc.dram_tensor("bq_s", [128, KC], F32, kind="ExternalInput")
    bk_s = nc.dram_tensor("bk_s", [128, KC], F32, kind="ExternalInput")
    bv_r = nc.dram_tensor("bv_r", [1, D], F32, kind="ExternalInput")
    bo_s = nc.dram_tensor("bo_s", [128, KC], F32, kind="ExternalInput")
    b1_s = nc.dram_tensor("b1_s", [128, FM], F32, kind="ExternalInput")
    b2_s = nc.dram_tensor("b2_s", [128, KC], F32, kind="ExternalInput")
    wqsum_s = nc.dram_tensor("wqsum_s", [128, KC], F32, kind="ExternalInput")
    wksum_s = nc.dram_tensor("wksum_s", [128, KC], F32, kind="ExternalInput")
    wvsum_r = nc.dram_tensor("wvsum_r", [1, D], F32, kind="ExternalInput")
    w1sum_s = nc.dram_tensor("w1sum_s", [128, FM], F32, kind="ExternalInput")
    msel = nc.dram_tensor("msel", [1, 4], F32, kind="ExternalInput")
    out_t = nc.dram_tensor("out_t", [D, T], F32, kind="ExternalOutput")

    rg_all = [list(range(NCORES))]
    rg_grp = [[0, 1, 2, 3], [4, 5, 6, 7]]
    AF = mybir.ActivationFunctionType
    ALU = mybir.AluOpType

    with tile.TileContext(nc) as tc, ExitStack() as ctx:
        const = ctx.enter_context(tc.tile_pool(name="const", bufs=1))
        tiny = ctx.enter_context(tc.tile_pool(name="tiny", bufs=1))
        ps = ctx.enter_context(tc.tile_pool(name="ps", bufs=2, space="PSUM"))
        dram = ctx.enter_context(tc.tile_pool(name="dram", bufs=1, space="DRAM"))

        # dram bounce buffers
        ar1_in = dram.tile([1, 4], F32, tag="ar1i")
        ar1_out = dram.tile([1, 4], F32, tag="ar1o")
        ar2_in = dram.tile([1, 4], F32, tag="ar2i")
        ar2_out = dram.tile([1, 4], F32, tag="ar2o")
        q_in = dram.tile([1, Q_ELEMS], BF16, tag="qin")
        q_out = dram.tile([1, RANKS * Q_ELEMS], BF16, tag="qout")
        rs_ins = [
            dram.tile([1, RANKS * HG * (DH + 1) * T], BF16, tag=f"rsi{g}",
                      name=f"rsi{g}")
            for g in range(NRS)
        ]
        rs_outs = [
            dram.tile([1, HG * (DH + 1) * T], BF16, tag=f"rso{g}",
                      name=f"rso{g}")
            for g in range(NRS)
        ]

        # ---- phase A: x load + LN1 partial sums ----
        x_T = const.tile([128, KC, T], F32, tag="xT")
        nc.sync.dma_start(
            out=x_T, in_=x_t.ap().rearrange("(kc p) t -> p kc t", p=128)
        )

        bq_sb = const.tile([128, KC], F32, tag="bq")
        bk_sb = const.tile([128, KC], F32, tag="bk")
        bo_sb = const.tile([128, KC], F32, tag="bo")
        b1_sb = const.tile([128, FM], F32, tag="b1")
        b2_sb = const.tile([128, KC], F32, tag="b2")
        wqsum_sb = const.tile([128, KC], F32, tag="wqs")
        wksum_sb = const.tile([128, KC], F32, tag="wks")
        w1sum_sb = const.tile([128, FM], F32, tag="w1s")
        nc.sync.dma_start(out=bq_sb, in_=bq_s.ap())
        nc.sync.dma_start(out=bk_sb, in_=bk_s.ap())
        nc.sync.dma_start(out=bo_sb, in_=bo_s.ap())
        nc.sync.dma_start(out=b1_sb, in_=b1_s.ap())
        nc.sync.dma_start(out=b2_sb, in_=b2_s.ap())
        nc.sync.dma_start(out=wqsum_sb, in_=wqsum_s.ap())
        nc.sync.dma_start(out=wksum_sb, in_=wksum_s.ap())
        nc.sync.dma_start(out=w1sum_sb, in_=w1sum_s.ap())
        msel_sb = const.tile([1, 4], F32, tag="msel")
        nc.sync.dma_start(out=msel_sb, in_=msel.ap())
        eps_t = const.tile([1, 1], F32, tag="eps")
        nc.vector.memset(eps_t, EPS)
        ones = const.tile([128, 1], F32, tag="ones")
        nc.vector.memset(ones, 1.0)

        xb = const.tile([128, KC, T], BF16, tag="xb")
        nc.vector.tensor_copy(out=xb, in_=x_T)
        _ln_stats_pre(nc, const, tiny, ps, x_T, msel_sb, ones, ar1_in, "ln1")

        with tc.tile_pool(name="po", bufs=1) as po_pool, \
             tc.tile_pool(name="pd", bufs=1) as pd:
            o_T = po_pool.tile([128, KC, T], BF16, tag="oT")
            wo_sb = pd.tile([128, KC, D], BF16, tag="wo")
            nc.scalar.dma_start(
                out=wo_sb,
                in_=wo_t.ap().rearrange("(kc p) n -> p kc n", p=128),
            )

            with tc.tile_pool(name="pq", bufs=1) as pq:
                k_loc = pq.tile([128, HP, T], BF16, tag="k_loc")
                v_send = pq.tile([128, 4, H, DH + 1], BF16, tag="v_send")
                qpre = pq.tile([128, HP, T], BF16, tag="qpre")
                kpre = pq.tile([128, HP, T], BF16, tag="kpre")
                vpre = pq.tile([128, 4, H, DH], BF16, tag="vpre")

                # ---- phase B: QKV on raw x. Q first: its (pre-fixup)
                # AllGather is the first collective on the stream, so the
                # bootstrap barrier burns during the K/V matmuls. ----
                with tc.tile_pool(name="wpool", bufs=2) as wp, \
                     tc.tile_pool(name="psq", bufs=4, space="PSUM") as psq:
                    for wdram, dst in ((wq_t, qpre), (wk_t, kpre)):
                        w_sb = wp.tile([128, KC, D], BF16, tag="w")
                        nc.sync.dma_start(
                            out=w_sb,
                            in_=wdram.ap().rearrange(
                                "(kc p) n -> p kc n", p=128
                            ),
                        )
                        for hp in range(HP):
                            pt = psq.tile([128, T], F32, tag="psq")
                            for kc in range(KC):
                                nc.tensor.matmul(
                                    pt,
                                    w_sb[:, kc, hp * 128:(hp + 1) * 128],
                                    xb[:, kc, :],
                                    start=(kc == 0),
                                    stop=(kc == KC - 1),
                                )
                            nc.scalar.copy(out=dst[:, hp, :], in_=pt)
                        if dst is qpre:
                            nc.scalar.dma_start(
                                out=_ap(
                                    q_in, 0, [(HP * T, 128), (T, HP), (1, T)]
                                ),
                                in_=qpre,
                            )
                            nc.gpsimd.collective_compute(
                                "AllGather", mybir.AluOpType.bypass,
                                replica_groups=rg_grp,
                                ins=[q_in.opt()], outs=[q_out.opt()],
                            )
                            # LN1 AllReduce queues on the stream behind
                            # the AllGather; result needed only for fixups
                            mu1, rs1, nrsmu1 = _ln_stats_post(
                                nc, tiny, msel_sb, eps_t, ar1_in, ar1_out,
                                rg_all, "ln1",
                            )
                            cbq = const.tile([128, KC], F32, tag="cbq")
                            nc.vector.tensor_scalar(
                                out=cbq, in0=wqsum_sb, scalar1=nrsmu1,
                                scalar2=None, op0=ALU.mult,
                            )
                            nc.vector.tensor_tensor(
                                out=cbq, in0=cbq, in1=bq_sb, op=ALU.add
                            )
                            cbk = const.tile([128, KC], F32, tag="cbk")
                            nc.vector.tensor_scalar(
                                out=cbk, in0=wksum_sb, scalar1=nrsmu1,
                                scalar2=None, op0=ALU.mult,
                            )
                            nc.vector.tensor_tensor(
                                out=cbk, in0=cbk, in1=bk_sb, op=ALU.add
                            )

                    wv_sb = wp.tile([128, KC, D], BF16, tag="w")
                    nc.sync.dma_start(
                        out=wv_sb,
                        in_=wv_t.ap().rearrange("(kc p) n -> p kc n", p=128),
                    )
                    # per-column V fixup constants (cvb = bv - rs*mu*wvsum)
                    bv_bc = wp.tile([128, D], F32, tag="bv", bufs=1)
                    nc.sync.dma_start(
                        out=bv_bc, in_=bv_r.ap().to_broadcast((128, D))
                    )
                    wvsum_bc = wp.tile([128, D], F32, tag="wvs", bufs=1)
                    nc.sync.dma_start(
                        out=wvsum_bc, in_=wvsum_r.ap().to_broadcast((128, D))
                    )
                    cvb = wp.tile([128, D], F32, tag="cvb", bufs=1)
                    nc.vector.tensor_scalar(
                        out=cvb, in0=wvsum_bc, scalar1=nrsmu1, scalar2=None,
                        op0=ALU.mult,
                    )
                    nc.vector.tensor_tensor(
                        out=cvb, in0=cvb, in1=bv_bc, op=ALU.add
                    )
                    for tcnk in range(4):
                        for n in range(2):
                            pt = psq.tile([128, 512], F32, tag="psq")
                            for kc in range(KC):
                                nc.tensor.matmul(
                                    pt,
                                    xb[:, kc, tcnk * 128:(tcnk + 1) * 128],
                                    wv_sb[:, kc, n * 512:(n + 1) * 512],
                                    start=(kc == 0),
                                    stop=(kc == KC - 1),
                                )
                            nc.scalar.copy(
                                out=vpre[:, tcnk, n * 8:(n + 1) * 8, :],
                                in_=pt.rearrange("p (h d) -> p h d", d=DH),
                            )
                    # k / v fixups
                    for hp in range(HP):
                        nc.vector.tensor_scalar(
                            out=k_loc[:, hp, :], in0=kpre[:, hp, :],
                            scalar1=rs1, scalar2=cbk[:, hp:hp + 1],
                            op0=ALU.mult, op1=ALU.add,
                        )
                    for tcnk in range(4):
                        vtmp = wp.tile([128, H, DH], BF16, tag="vtmp")
                        nc.vector.tensor_scalar(
                            out=vtmp, in0=vpre[:, tcnk, :, :],
                            scalar1=rs1, scalar2=None, op0=ALU.mult,
                        )
                        nc.vector.tensor_tensor(
                            out=v_send[:, tcnk, :, 0:DH],
                            in0=vtmp,
                            in1=cvb.rearrange("p (h d) -> p h d", d=DH),
                            op=ALU.add,
                        )
                    nc.vector.memset(v_send[:, :, :, DH:DH + 1], 1.0)

                # ---- phase C: attention (all queries x local keys) ----
                with tc.tile_pool(name="att", bufs=1) as patt, \
                     tc.tile_pool(name="etp", bufs=3) as etp, \
                     tc.tile_pool(name="nmp", bufs=2) as nmp, \
                     tc.tile_pool(name="ps_s", bufs=2, space="PSUM") as ps_s, \
                     tc.tile_pool(name="ps_o", bufs=2, space="PSUM") as ps_o:
                    qf = []
                    for r in range(RANKS):
                        qa = patt.tile([128, HP, T], BF16, tag=f"qa{r}",
                                       name=f"qa{r}")
                        nc.sync.dma_start(
                            out=qa,
                            in_=_ap(
                                q_out, r * Q_ELEMS,
                                [(HP * T, 128), (T, HP), (1, T)],
                            ),
                        )
                        # LN1 fixup in place on the gathered pre-Q: same
                        # batch stats on all ranks; cbq varies per partition
                        # and per hp chunk
                        for hp in range(HP):
                            nc.vector.tensor_scalar(
                                out=qa[:, hp, :], in0=qa[:, hp, :],
                                scalar1=rs1, scalar2=cbq[:, hp:hp + 1],
                                op0=ALU.mult, op1=ALU.add,
                            )
                        qf.append(qa)

                    def normalize_group(g):
                        na = nmp.tile([DH + 1, HG, T], BF16, tag="na",
                                      name="na")
                        nc.sync.dma_start(
                            out=na,
                            in_=_ap(
                                rs_outs[g], 0,
                                [(T, DH + 1), ((DH + 1) * T, HG), (1, T)],
                            ),
                        )
                        for hh in range(HG):
                            h = g * HG + hh
                            den = nmp.tile([1, T], BF16, tag="den",
                                           name="den")
                            nc.gpsimd.dma_start(
                                out=den, in_=na[DH:DH + 1, hh, :]
                            )
                            rec = nmp.tile([1, T], F32, tag="rec", name="rec")
                            nc.vector.reciprocal(out=rec, in_=den)
                            rb = nmp.tile([DH, T], F32, tag="rb", name="rb")
                            nc.gpsimd.partition_broadcast(rb, rec, channels=DH)
                            stg = nmp.tile([DH, T], BF16, tag="stg",
                                           name="stg")
                            nc.vector.tensor_tensor(
                                out=stg, in0=na[0:DH, hh, :], in1=rb,
                                op=ALU.mult,
                            )
                            nc.sync.dma_start(
                                out=o_T[(h % 2) * DH:(h % 2 + 1) * DH,
                                        h // 2, :],
                                in_=stg,
                            )

                    for h in range(H):
                        hp, off = h // 2, (h % 2) * DH
                        for r in range(RANKS):
                            po = ps_o.tile([DH + 1, T], F32, tag="ps_o")
                            for kk in (0, 2):
                                pss = ps_s.tile([128, 2, T], F32, tag="ps_s")
                                for j in range(2):
                                    kc = kk + j
                                    nc.tensor.matmul(
                                        pss[:, j, :],
                                        k_loc[off:off + DH, hp,
                                              kc * 128:(kc + 1) * 128],
                                        qf[r][off:off + DH, hp, :],
                                        start=True,
                                        stop=True,
                                    )
                                et = etp.tile([128, 2, T], BF16, tag="et")
                                nc.scalar.activation(
                                    out=et, in_=pss, func=AF.Exp,
                                    scale=float(SCALE),
                                )
                                for j in range(2):
                                    kc = kk + j
                                    nc.tensor.matmul(
                                        po,
                                        v_send[:, kc, h, :],
                                        et[:, j, :],
                                        start=(kc == 0),
                                        stop=(kc == 3),
                                    )
                            num = nmp.tile([DH + 1, T], BF16, tag="num")
                            nc.vector.tensor_copy(out=num, in_=po)
                            g = h // HG
                            nc.gpsimd.dma_start(
                                out=_ap(
                                    rs_ins[g],
                                    (r * HG + (h % HG)) * (DH + 1) * T,
                                    [(T, DH + 1), (1, T)],
                                ),
                                in_=num,
                            )
                        if h % HG == HG - 1:
                            g = h // HG
                            nc.gpsimd.collective_compute(
                                "ReduceScatter", mybir.AluOpType.add,
                                replica_groups=rg_grp,
                                ins=[rs_ins[g].opt()],
                                outs=[rs_outs[g].opt()],
                            )
                            if g >= 1:
                                normalize_group(g - 1)
                    normalize_group(NRS - 1)

            # ---- phase D: o_proj + residual + LN2 ----
            with tc.tile_pool(name="pdt", bufs=2) as pdt:
                r_T = pd.tile([128, KC, T], F32, tag="rT")
                rbf = pd.tile([128, KC, T], BF16, tag="rbf")
                for m in range(KC):
                    pt = ps.tile([128, 512], F32, tag="ps")
                    for kc in range(KC):
                        nc.tensor.matmul(
                            pt,
                            wo_sb[:, kc, m * 128:(m + 1) * 128],
                            o_T[:, kc, :],
                            start=(kc == 0),
                            stop=(kc == KC - 1),
                        )
                    tmp = pdt.tile([128, T], F32, tag="otmp")
                    nc.scalar.activation(
                        out=tmp, in_=pt, func=AF.Identity,
                        bias=bo_sb[:, m:m + 1],
                    )
                    nc.vector.tensor_tensor(
                        out=r_T[:, m, :], in0=tmp, in1=x_T[:, m, :],
                        op=ALU.add,
                    )
                    nc.vector.tensor_copy(out=rbf[:, m, :], in_=r_T[:, m, :])
                _ln_stats_pre(
                    nc, const, tiny, ps, r_T, msel_sb, ones, ar2_in, "ln2"
                )
                mu2, rs2, nrsmu2 = _ln_stats_post(
                    nc, tiny, msel_sb, eps_t, ar2_in, ar2_out, rg_all, "ln2"
                )
                cb1 = const.tile([128, FM], F32, tag="cb1")
                nc.vector.tensor_scalar(
                    out=cb1, in0=w1sum_sb, scalar1=nrsmu2, scalar2=None,
                    op0=ALU.mult,
                )
                nc.vector.tensor_tensor(out=cb1, in0=cb1, in1=b1_sb, op=ALU.add)

                # ---- phase E: FFN (FFN1 on raw r; LN2 folded into the
                # Relu activation's scale/bias; deep PSUM pool rides out
                # the AllReduce latency) ----
                with tc.tile_pool(name="ffn", bufs=1) as pffn, \
                     tc.tile_pool(name="w1p", bufs=2) as w1p, \
                     tc.tile_pool(name="w2p", bufs=2) as w2p, \
                     tc.tile_pool(name="fout", bufs=2) as fop, \
                     tc.tile_pool(name="psf", bufs=6, space="PSUM") as psf:
                    f_T = pffn.tile([128, FM, T], BF16, tag="fT")
                    for m in range(FM):
                        w1c = w1p.tile([128, KC, 128], BF16, tag="w1c")
                        nc.sync.dma_start(
                            out=w1c,
                            in_=w1_t.ap()[:, m * 128:(m + 1) * 128]
                            .rearrange("(kc p) n -> p kc n", p=128),
                        )
                        pt = psf.tile([128, 512], F32, tag="psf")
                        for kc in range(KC):
                            nc.tensor.matmul(
                                pt, w1c[:, kc, :], rbf[:, kc, :],
                                start=(kc == 0), stop=(kc == KC - 1),
                            )
                        nc.scalar.activation(
                            out=f_T[:, m, :], in_=pt, func=AF.Relu,
                            bias=cb1[:, m:m + 1], scale=rs2,
                        )

                    for m in range(KC):
                        w2c = w2p.tile([128, FM, 128], BF16, tag="w2c")
                        nc.sync.dma_start(
                            out=w2c,
                            in_=w2_t.ap()[:, m * 128:(m + 1) * 128]
                            .rearrange("(kc p) n -> p kc n", p=128),
                        )
                        pt = psf.tile([128, 512], F32, tag="psf")
                        for kc in range(FM):
                            nc.tensor.matmul(
                                pt, w2c[:, kc, :], f_T[:, kc, :],
                                start=(kc == 0), stop=(kc == FM - 1),
                            )
                        tmp = fop.tile([128, T], F32, tag="ftmp")
                        nc.scalar.activation(
                            out=tmp, in_=pt, func=AF.Identity,
                            bias=b2_sb[:, m:m + 1],
                        )
                        fin = fop.tile([128, T], F32, tag="fin")
                        nc.vector.tensor_tensor(
                            out=fin, in0=tmp, in1=x_T[:, m, :], op=ALU.add
                        )
                        nc.sync.dma_start(
                            out=out_t.ap().rearrange(
                                "(kc p) t -> p kc t", p=128
                            )[:, m, :],
                            in_=fin,
                        )

    nc.compile()
    return nc


def _get_nc():
    if "nc" not in _CACHE:
        _CACHE["nc"] = _build()
    return _CACHE["nc"]


def _prep_in_maps(inputs):
    x = np.asarray(inputs["x"], np.float32)
    common = {}
    ws = {}
    for name in ("wq", "wk", "wv", "wo", "w1", "w2"):
        ws[name] = np.asarray(inputs[name], np.float32)
        common[name + "_t"] = np.ascontiguousarray(
            ws[name].T
        ).astype(ml_dtypes.bfloat16)
    bq = np.asarray(inputs["bq"], np.float32)
    bk = np.asarray(inputs["bk"], np.float32)
    bv = np.asarray(inputs["bv"], np.float32)
    bo = np.asarray(inputs["bo"], np.float32)
    b1 = np.asarray(inputs["b1"], np.float32)
    b2 = np.asarray(inputs["b2"], np.float32)
    common["bq_s"] = np.ascontiguousarray(bq.reshape(KC, 128).T)
    common["bk_s"] = np.ascontiguousarray(bk.reshape(KC, 128).T)
    common["bv_r"] = bv.reshape(1, D)
    common["bo_s"] = np.ascontiguousarray(bo.reshape(KC, 128).T)
    common["b1_s"] = np.ascontiguousarray(b1.reshape(FM, 128).T)
    common["b2_s"] = np.ascontiguousarray(b2.reshape(KC, 128).T)
    common["wqsum_s"] = np.ascontiguousarray(
        ws["wq"].sum(axis=1).reshape(KC, 128).T
    )
    common["wksum_s"] = np.ascontiguousarray(
        ws["wk"].sum(axis=1).reshape(KC, 128).T
    )
    common["wvsum_r"] = ws["wv"].sum(axis=1).reshape(1, D)
    common["w1sum_s"] = np.ascontiguousarray(
        ws["w1"].sum(axis=1).reshape(FM, 128).T
    )

    xf = x.reshape(B * L, D)
    in_maps = []
    for c in range(NCORES):
        m = dict(common)
        m["x_t"] = np.ascontiguousarray(xf[c * T:(c + 1) * T].T)
        m["msel"] = (
            np.array([[1, 1, 0, 0]], np.float32)
            if c // RANKS == 0
            else np.array([[0, 0, 1, 1]], np.float32)
        )
        in_maps.append(m)
    return in_maps


def _assemble(res):
    out = np.empty((B * L, D), np.float32)
    for c in range(NCORES):
        out[c * T:(c + 1) * T] = res.results[c]["out_t"].T
    return out.reshape(B, L, D)


def kernel(**inputs):
    nc = _get_nc()
    in_maps = _prep_in_maps(inputs)
    res = bass_utils.run_bass_kernel_spmd(
        nc, in_maps, core_ids=list(range(NCORES))
    )
    return _assemble(res)


def traced_run(inputs):
    nc = _get_nc()
    in_maps = _prep_in_maps(inputs)
    return bass_utils.run_bass_kernel_spmd(
        nc, in_maps, core_ids=list(range(NCORES)), trace=True
    )


# revision 15
# speedup vs baseline: 1.0876x; 1.0876x over previous
"""Trainium2 Bass kernel for nn_EncoderBlock (B=2, L=2048, D=1024, H=16, FF=4096).

Sharding: sequence-parallel over the 4096 tokens across 8 cores (512 tokens
per core; cores 0-3 own batch 0, cores 4-7 own batch 1). Per-core work is
dense (full weights). Collectives (all overlapped with compute):
  - AllGather (4-core groups) of the PRE-normalization Q projection (bf16),
    triggered first so the runtime's collective bootstrap barrier and the
    NEFF-launch skew burn during the K/V matmuls. The LN fixup is affine
    and per-partition with identical stats across the group, so it is
    applied to the gathered result instead.
  - AllReduce (8 cores) of masked LayerNorm partial sums (LN1, LN2), 16B
    each. QKV / FFN1 matmuls run on the RAW input concurrently; the
    normalization is applied afterwards as an affine fixup:
    W @ ((x - mu) * rs) = rs * (W @ x) - rs * mu * rowsum(W).
  - 8x ReduceScatter (4-core groups) of partial attention numerators and
    softmax denominators (additive across key shards); each core receives
    the full-key sums for its own 512 queries. Pipelined per 2-head group,
    with normalization interleaved one group behind the triggers.

Each core scores ALL 2048 queries of its batch against its LOCAL 512
keys/values. Activations keep the feature dim on partitions and tokens on
the free axis so matmuls contract along partitions with no transposes.
Weights are pre-transposed and cast to bf16 on the host. Softmax
denominators come from a ones-column appended to V (row 64 of the att@v
accumulator).

Dtypes: bf16 operands for all matmuls; fp32 stats/residuals/PSUM accum.
"""

import sys

sys.path.insert(0, "/opt/trn_rl_repo")

from contextlib import ExitStack  # noqa: E402

import numpy as np  # noqa: E402
import ml_dtypes  # noqa: E402

import concourse.bass as bass  # noqa: E402
import concourse.mybir as mybir  # noqa: E402
import concourse.tile as tile  # noqa: E402
from concourse import bacc, bass_utils  # noqa: E402

B, L, D, H, FF = 2, 2048, 1024, 16, 4096
DH = D // H  # 64
NCORES = 8
RANKS = 4  # cores per batch group
T = B * L // NCORES  # 512 tokens per core
KC = D // 128  # 8 contraction chunks of 128
HP = H // 2  # 8 head-pairs (2 heads per 128-partition chunk)
FM = FF // 128  # 32 ff chunks
NTOT = float(L * D)  # layernorm element count per batch
EPS = 1e-5
SCALE = 1.0 / np.sqrt(np.float32(H))  # faithful to source bug: 1/sqrt(H)

F32 = mybir.dt.float32
BF16 = mybir.dt.bfloat16

Q_ELEMS = 128 * HP * T  # qpre [128, 8, 512] bf16
HG = 2  # heads per ReduceScatter group
NRS = H // HG  # 8 ReduceScatter ops

_CACHE = {}


def _ap(t, offset, dims):
    """Manual AP over a dram tile: dims = [(step, count), ...], partition first."""
    return bass.AP(
        tensor=t.tensor, offset=t.offset + offset, ap=[[s, c] for s, c in dims]
    )


def _ln_stats_pre(nc, const, tiny, ps, src, msel_sb, ones, ar_in, pfx):
    """Partial LN sums of src -> masked [1,4] staged in ar_in (DRAM)."""
    AF = mybir.ActivationFunctionType
    s_part = tiny.tile([128, 1], F32, tag=pfx + "_s")
    nc.vector.tensor_reduce(
        out=s_part, in_=src, axis=mybir.AxisListType.XY, op=mybir.AluOpType.add
    )
    junk = const.tile([128, KC, T], BF16, tag="junk")
    q_part = tiny.tile([128, 1], F32, tag=pfx + "_q")
    nc.scalar.activation(out=junk, in_=src, func=AF.Square, accum_out=q_part)
    st2 = tiny.tile([128, 2], F32, tag=pfx + "_st2")
    nc.vector.tensor_copy(out=st2[:, 0:1], in_=s_part)
    nc.vector.tensor_copy(out=st2[:, 1:2], in_=q_part)
    ps_st = ps.tile([1, 2], F32, tag="ps")
    nc.tensor.matmul(ps_st, ones, st2, start=True, stop=True)
    sb4 = tiny.tile([1, 4], F32, tag=pfx + "_sb4")
    nc.scalar.copy(out=sb4[0:1, 0:2], in_=ps_st)
    nc.scalar.copy(out=sb4[0:1, 2:4], in_=ps_st)
    nc.vector.tensor_mul(out=sb4, in0=sb4, in1=msel_sb)
    nc.sync.dma_start(out=ar_in, in_=sb4)


def _ln_stats_post(nc, tiny, msel_sb, eps_t, ar_in, ar_out, rg_all, pfx):
    """AllReduce the staged sums; derive (mu_b, rs_b, nrsmu_b) [128,1]."""
    AF = mybir.ActivationFunctionType
    nc.gpsimd.collective_compute(
        "AllReduce", mybir.AluOpType.add, replica_groups=rg_all,
        ins=[ar_in.opt()], outs=[ar_out.opt()],
    )
    r4 = tiny.tile([1, 4], F32, tag=pfx + "_r4")
    nc.sync.dma_start(out=r4, in_=ar_out)
    nc.vector.tensor_mul(out=r4, in0=r4, in1=msel_sb)
    sq2 = tiny.tile([1, 2], F32, tag=pfx + "_sq2")
    nc.vector.tensor_tensor(
        out=sq2, in0=r4[0:1, 0:2], in1=r4[0:1, 2:4], op=mybir.AluOpType.add
    )
    mean = tiny.tile([1, 1], F32, tag=pfx + "_mean")
    nc.scalar.mul(out=mean, in_=sq2[0:1, 0:1], mul=1.0 / NTOT)
    e2 = tiny.tile([1, 1], F32, tag=pfx + "_e2")
    nc.scalar.mul(out=e2, in_=sq2[0:1, 1:2], mul=1.0 / NTOT)
    musq = tiny.tile([1, 1], F32, tag=pfx + "_musq")
    nc.vector.tensor_mul(out=musq, in0=mean, in1=mean)
    var = tiny.tile([1, 1], F32, tag=pfx + "_var")
    nc.vector.tensor_tensor(
        out=var, in0=e2, in1=musq, op=mybir.AluOpType.subtract
    )
    sd = tiny.tile([1, 1], F32, tag=pfx + "_sd")
    nc.scalar.activation(out=sd, in_=var, func=AF.Sqrt, bias=eps_t)
    rs = tiny.tile([1, 1], F32, tag=pfx + "_rs")
    nc.vector.reciprocal(out=rs, in_=sd)
    rsmu = tiny.tile([1, 1], F32, tag=pfx + "_rsmu")
    nc.vector.tensor_mul(out=rsmu, in0=mean, in1=rs)
    nrsmu = tiny.tile([1, 1], F32, tag=pfx + "_nrsmu")
    nc.scalar.mul(out=nrsmu, in_=rsmu, mul=-1.0)
    mu_b = tiny.tile([128, 1], F32, tag=pfx + "_mub")
    rs_b = tiny.tile([128, 1], F32, tag=pfx + "_rsb")
    nrsmu_b = tiny.tile([128, 1], F32, tag=pfx + "_nmb")
    nc.gpsimd.partition_broadcast(mu_b, mean)
    nc.gpsimd.partition_broadcast(rs_b, rs)
    nc.gpsimd.partition_broadcast(nrsmu_b, nrsmu)
    return mu_b, rs_b, nrsmu_b


def _build():
    nc = bacc.Bacc("TRN2", target_bir_lowering=False, debug=False,
                   num_devices=NCORES)

    x_t = nc.dram_tensor("x_t", [D, T], F32, kind="ExternalInput")
    wq_t = nc.dram_tensor("wq_t", [D, D], BF16, kind="ExternalInput")
    wk_t = nc.dram_tensor("wk_t", [D, D], BF16, kind="ExternalInput")
    wv_t = nc.dram_tensor("wv_t", [D, D], BF16, kind="ExternalInput")
    wo_t = nc.dram_tensor("wo_t", [D, D], BF16, kind="ExternalInput")
    w1_t = nc.dram_tensor("w1_t", [D, FF], BF16, kind="ExternalInput")
    w2_t = nc.dram_tensor("w2_t", [FF, D], BF16, kind="ExternalInput")
    bq_s = nc.dram_tensor("bq_s", [128, KC], F32, kind="ExternalInput")
    bk_s = nc.dram_tensor("bk_s", [128, KC], F32, kind="ExternalInput")
    bv_r = nc.dram_tensor("bv_r", [1, D], F32, kind="ExternalInput")
    bo_s = nc.dram_tensor("bo_s", [128, KC], F32, kind="ExternalInput")
    b1_s = nc.dram_tensor("b1_s", [128, FM], F32, kind="ExternalInput")
    b2_s = nc.dram_tensor("b2_s", [128, KC], F32, kind="ExternalInput")
    wqsum_s = nc.dram_tensor("wqsum_s", [128, KC], F32, kind="ExternalInput")
    wksum_s = nc.dram_tensor("wksum_s", [128, KC], F32, kind="ExternalInput")
    wvsum_r = nc.dram_tensor("wvsum_r", [1, D], F32, kind="ExternalInput")
    w1sum_s = nc.dram_tensor("w1sum_s", [128, FM], F32, kind="ExternalInput")
    msel = nc.dram_tensor("msel", [1, 4], F32, kind="ExternalInput")
    out_t = nc.dram_tensor("out_t", [D, T], F32, kind="ExternalOutput")

    rg_all = [list(range(NCORES))]
    rg_grp = [[0, 1, 2, 3], [4, 5, 6, 7]]
    AF = mybir.ActivationFunctionType
    ALU = mybir.AluOpType

    with tile.TileContext(nc) as tc, ExitStack() as ctx:
        const = ctx.enter_context(tc.tile_pool(name="const", bufs=1))
        tiny = ctx.enter_context(tc.tile_pool(name="tiny", bufs=1))
        ps = ctx.enter_context(tc.tile_pool(name="ps", bufs=2, space="PSUM"))
        dram = ctx.enter_context(tc.tile_pool(name="dram", bufs=1, space="DRAM"))

        # dram bounce buffers
        ar1_in = dram.tile([1, 4], F32, tag="ar1i")
        ar1_out = dram.tile([1, 4], F32, tag="ar1o")
        ar2_in = dram.tile([1, 4], F32, tag="ar2i")
        ar2_out = dram.tile([1, 4], F32, tag="ar2o")
        q_in = dram.tile([1, Q_ELEMS], BF16, tag="qin")
        q_out = dram.tile([1, RANKS * Q_ELEMS], BF16, tag="qout")
        rs_ins = [
            dram.tile([1, RANKS * HG * (DH + 1) * T], BF16, tag=f"rsi{g}",
                      name=f"rsi{g}")
            for g in range(NRS)
        ]
        rs_outs = [
            dram.tile([1, HG * (DH + 1) * T], BF16, tag=f"rso{g}",
                      name=f"rso{g}")
            for g in range(NRS)
        ]

        # ---- phase A: x load + LN1 partial sums ----
        x_T = const.tile([128, KC, T], F32, tag="xT")
        nc.sync.dma_start(
            out=x_T, in_=x_t.ap().rearrange("(kc p) t -> p kc t", p=128)
        )

        bq_sb = const.tile([128, KC], F32, tag="bq")
        bk_sb = const.tile([128, KC], F32, tag="bk")
        bo_sb = const.tile([128, KC], F32, tag="bo")
        b1_sb = const.tile([128, FM], F32, tag="b1")
        b2_sb = const.tile([128, KC], F32, tag="b2")
        wqsum_sb = const.tile([128, KC], F32, tag="wqs")
        wksum_sb = const.tile([128, KC], F32, tag="wks")
        w1sum_sb = const.tile([128, FM], F32, tag="w1s")
        nc.sync.dma_start(out=bq_sb, in_=bq_s.ap())
        nc.sync.dma_start(out=bk_sb, in_=bk_s.ap())
        nc.sync.dma_start(out=bo_sb, in_=bo_s.ap())
        nc.sync.dma_start(out=b1_sb, in_=b1_s.ap())
        nc.sync.dma_start(out=b2_sb, in_=b2_s.ap())
        nc.sync.dma_start(out=wqsum_sb, in_=wqsum_s.ap())
        nc.sync.dma_start(out=wksum_sb, in_=wksum_s.ap())
        nc.sync.dma_start(out=w1sum_sb, in_=w1sum_s.ap())
        msel_sb = const.tile([1, 4], F32, tag="msel")
        nc.sync.dma_start(out=msel_sb, in_=msel.ap())
        eps_t = const.tile([1, 1], F32, tag="eps")
        nc.vector.memset(eps_t, EPS)
        ones = const.tile([128, 1], F32, tag="ones")
        nc.vector.memset(ones, 1.0)

        xb = const.tile([128, KC, T], BF16, tag="xb")
        nc.vector.tensor_copy(out=xb, in_=x_T)
        _ln_stats_pre(nc, const, tiny, ps, x_T, msel_sb, ones, ar1_in, "ln1")

        with tc.tile_pool(name="po", bufs=1) as po_pool, \
             tc.tile_pool(name="pd", bufs=1) as pd:
            o_T = po_pool.tile([128, KC, T], BF16, tag="oT")
            wo_sb = pd.tile([128, KC, D], BF16, tag="wo")
            nc.scalar.dma_start(
                out=wo_sb,
                in_=wo_t.ap().rearrange("(kc p) n -> p kc n", p=128),
            )

            with tc.tile_pool(name="pq", bufs=1) as pq:
                k_loc = pq.tile([128, HP, T], BF16, tag="k_loc")
                v_send = pq.tile([128, 4, H, DH + 1], BF16, tag="v_send")
                qpre = pq.tile([128, HP, T], BF16, tag="qpre")
                kpre = pq.tile([128, HP, T], BF16, tag="kpre")
                vpre = pq.tile([128, 4, H, DH], BF16, tag="vpre")

                # ---- phase B: QKV on raw x. Q first: its (pre-fixup)
                # AllGather is the first collective on the stream, so the
                # bootstrap barrier burns during the K/V matmuls. ----
                with tc.tile_pool(name="wpool", bufs=2) as wp, \
                     tc.tile_pool(name="psq", bufs=4, space="PSUM") as psq:
                    for wdram, dst in ((wq_t, qpre), (wk_t, kpre)):
                        w_sb = wp.tile([128, KC, D], BF16, tag="w")
                        nc.sync.dma_start(
                            out=w_sb,
                            in_=wdram.ap().rearrange(
                                "(kc p) n -> p kc n", p=128
                            ),
                        )
                        for hp in range(HP):
                            pt = psq.tile([128, T], F32, tag="psq")
                            for kc in range(KC):
                                nc.tensor.matmul(
                                    pt,
                                    w_sb[:, kc, hp * 128:(hp + 1) * 128],
                                    xb[:, kc, :],
                                    start=(kc == 0),
                                    stop=(kc == KC - 1),
                                )
                            nc.scalar.copy(out=dst[:, hp, :], in_=pt)
                        if dst is qpre:
                            nc.scalar.dma_start(
                                out=_ap(
                                    q_in, 0, [(HP * T, 128), (T, HP), (1, T)]
                                ),
                                in_=qpre,
                            )
                            nc.gpsimd.collective_compute(
                                "AllGather", mybir.AluOpType.bypass,
                                replica_groups=rg_grp,
                                ins=[q_in.opt()], outs=[q_out.opt()],
                            )
                            # LN1 AllReduce queues on the stream behind
                            # the AllGather; result needed only for fixups
                            mu1, rs1, nrsmu1 = _ln_stats_post(
                                nc, tiny, msel_sb, eps_t, ar1_in, ar1_out,
                                rg_all, "ln1",
                            )
                            cbq = const.tile([128, KC], F32, tag="cbq")
                            nc.vector.tensor_scalar(
                                out=cbq, in0=wqsum_sb, scalar1=nrsmu1,
                                scalar2=None, op0=ALU.mult,
                            )
                            nc.vector.tensor_tensor(
                                out=cbq, in0=cbq, in1=bq_sb, op=ALU.add
                            )
                            cbk = const.tile([128, KC], F32, tag="cbk")
                            nc.vector.tensor_scalar(
                                out=cbk, in0=wksum_sb, scalar1=nrsmu1,
                                scalar2=None, op0=ALU.mult,
                            )
                            nc.vector.tensor_tensor(
                                out=cbk, in0=cbk, in1=bk_sb, op=ALU.add
                            )

                    wv_sb = wp.tile([128, KC, D], BF16, tag="w")
                    nc.sync.dma_start(
                        out=wv_sb,
                        in_=wv_t.ap().rearrange("(kc p) n -> p kc n", p=128),
                    )
                    # per-column V fixup constants (cvb = bv - rs*mu*wvsum)
                    bv_bc = wp.tile([128, D], F32, tag="bv", bufs=1)
                    nc.sync.dma_start(
                        out=bv_bc, in_=bv_r.ap().to_broadcast((128, D))
                    )
                    wvsum_bc = wp.tile([128, D], F32, tag="wvs", bufs=1)
                    nc.sync.dma_start(
                        out=wvsum_bc, in_=wvsum_r.ap().to_broadcast((128, D))
                    )
                    cvb = wp.tile([128, D], F32, tag="cvb", bufs=1)
                    nc.vector.tensor_scalar(
                        out=cvb, in0=wvsum_bc, scalar1=nrsmu1, scalar2=None,
                        op0=ALU.mult,
                    )
                    nc.vector.tensor_tensor(
                        out=cvb, in0=cvb, in1=bv_bc, op=ALU.add
                    )
                    for tcnk in range(4):
                        for n in range(2):
                            pt = psq.tile([128, 512], F32, tag="psq")
                            for kc in range(KC):
                                nc.tensor.matmul(
                                    pt,
                                    xb[:, kc, tcnk * 128:(tcnk + 1) * 128],
                                    wv_sb[:, kc, n * 512:(n + 1) * 512],
                                    start=(kc == 0),
                                    stop=(kc == KC - 1),
                                )
                            nc.scalar.copy(
                                out=vpre[:, tcnk, n * 8:(n + 1) * 8, :],
                                in_=pt.rearrange("p (h d) -> p h d", d=DH),
                            )
                    # k / v fixups
                    for hp in range(HP):
                        nc.vector.tensor_scalar(
                            out=k_loc[:, hp, :], in0=kpre[:, hp, :],
                            scalar1=rs1, scalar2=cbk[:, hp:hp + 1],
                            op0=ALU.mult, op1=ALU.add,
                        )
                    for tcnk in range(4):
                        vtmp = wp.tile([128, H, DH], BF16, tag="vtmp")
                        nc.vector.tensor_scalar(
                            out=vtmp, in0=vpre[:, tcnk, :, :],
                            scalar1=rs1, scalar2=None, op0=ALU.mult,
                        )
                        nc.vector.tensor_tensor(
                            out=v_send[:, tcnk, :, 0:DH],
                            in0=vtmp,
                            in1=cvb.rearrange("p (h d) -> p h d", d=DH),
                            op=ALU.add,
                        )
                    nc.vector.memset(v_send[:, :, :, DH:DH + 1], 1.0)

                # ---- phase C: attention (all queries x local keys) ----
                with tc.tile_pool(name="att", bufs=1) as patt, \
                     tc.tile_pool(name="etp", bufs=3) as etp, \
                     tc.tile_pool(name="nmp", bufs=3) as nmp, \
                     tc.tile_pool(name="ps_s", bufs=2, space="PSUM") as ps_s, \
                     tc.tile_pool(name="ps_o", bufs=2, space="PSUM") as ps_o:
                    qf = []
                    for r in range(RANKS):
                        qa = patt.tile([128, HP, T], BF16, tag=f"qa{r}",
                                       name=f"qa{r}")
                        nc.sync.dma_start(
                            out=qa,
                            in_=_ap(
                                q_out, r * Q_ELEMS,
                                [(HP * T, 128), (T, HP), (1, T)],
                            ),
                        )
                        # LN1 fixup in place on the gathered pre-Q: same
                        # batch stats on all ranks; cbq varies per partition
                        # and per hp chunk
                        for hp in range(HP):
                            nc.vector.tensor_scalar(
                                out=qa[:, hp, :], in0=qa[:, hp, :],
                                scalar1=rs1, scalar2=cbq[:, hp:hp + 1],
                                op0=ALU.mult, op1=ALU.add,
                            )
                        qf.append(qa)

                    def normalize_group(g):
                        na = nmp.tile([DH + 1, HG, T], BF16, tag="na",
                                      name="na")
                        nc.sync.dma_start(
                            out=na,
                            in_=_ap(
                                rs_outs[g], 0,
                                [(T, DH + 1), ((DH + 1) * T, HG), (1, T)],
                            ),
                        )
                        for hh in range(HG):
                            h = g * HG + hh
                            den = nmp.tile([1, T], BF16, tag="den",
                                           name="den")
                            nc.gpsimd.dma_start(
                                out=den, in_=na[DH:DH + 1, hh, :]
                            )
                            rec = nmp.tile([1, T], F32, tag="rec", name="rec")
                            nc.vector.reciprocal(out=rec, in_=den)
                            rb = nmp.tile([DH, T], F32, tag="rb", name="rb")
                            nc.gpsimd.partition_broadcast(rb, rec, channels=DH)
                            stg = nmp.tile([DH, T], BF16, tag="stg",
                                           name="stg")
                            nc.vector.tensor_tensor(
                                out=stg, in0=na[0:DH, hh, :], in1=rb,
                                op=ALU.mult,
                            )
                            nc.sync.dma_start(
                                out=o_T[(h % 2) * DH:(h % 2 + 1) * DH,
                                        h // 2, :],
                                in_=stg,
                            )

                    for h in range(H):
                        hp, off = h // 2, (h % 2) * DH
                        for r in range(RANKS):
                            po = ps_o.tile([DH + 1, T], F32, tag="ps_o")
                            for kk in (0, 2):
                                pss = ps_s.tile([128, 2, T], F32, tag="ps_s")
                                for j in range(2):
                                    kc = kk + j
                                    nc.tensor.matmul(
                                        pss[:, j, :],
                                        k_loc[off:off + DH, hp,
                                              kc * 128:(kc + 1) * 128],
                                        qf[r][off:off + DH, hp, :],
                                        start=True,
                                        stop=True,
                                    )
                                et = etp.tile([128, 2, T], BF16, tag="et")
                                nc.scalar.activation(
                                    out=et, in_=pss, func=AF.Exp,
                                    scale=float(SCALE),
                                )
                                for j in range(2):
                                    kc = kk + j
                                    nc.tensor.matmul(
                                        po,
                                        v_send[:, kc, h, :],
                                        et[:, j, :],
                                        start=(kc == 0),
                                        stop=(kc == 3),
                                    )
                            num = nmp.tile([DH + 1, T], BF16, tag="num")
                            nc.vector.tensor_copy(out=num, in_=po)
                            g = h // HG
                            nc.sync.dma_start(
                                out=_ap(
                                    rs_ins[g],
                                    (r * HG + (h % HG)) * (DH + 1) * T,
                                    [(T, DH + 1), (1, T)],
                                ),
                                in_=num,
                            )
                        if h % HG == HG - 1:
                            g = h // HG
                            nc.gpsimd.collective_compute(
                                "ReduceScatter", mybir.AluOpType.add,
                                replica_groups=rg_grp,
                                ins=[rs_ins[g].opt()],
                                outs=[rs_outs[g].opt()],
                            )
                            if g >= 2:
                                normalize_group(g - 2)
                    normalize_group(NRS - 2)
                    normalize_group(NRS - 1)

            # ---- phase D: o_proj + residual + LN2 ----
            with tc.tile_pool(name="pdt", bufs=2) as pdt:
                r_T = pd.tile([128, KC, T], F32, tag="rT")
                rbf = pd.tile([128, KC, T], BF16, tag="rbf")
                for m in range(KC):
                    pt = ps.tile([128, 512], F32, tag="ps")
                    for kc in range(KC):
                        nc.tensor.matmul(
                            pt,
                            wo_sb[:, kc, m * 128:(m + 1) * 128],
                            o_T[:, kc, :],
                            start=(kc == 0),
                            stop=(kc == KC - 1),
                        )
                    tmp = pdt.tile([128, T], F32, tag="otmp")
                    nc.scalar.activation(
                        out=tmp, in_=pt, func=AF.Identity,
                        bias=bo_sb[:, m:m + 1],
                    )
                    nc.vector.tensor_tensor(
                        out=r_T[:, m, :], in0=tmp, in1=x_T[:, m, :],
                        op=ALU.add,
                    )
                    nc.vector.tensor_copy(out=rbf[:, m, :], in_=r_T[:, m, :])
                _ln_stats_pre(
                    nc, const, tiny, ps, r_T, msel_sb, ones, ar2_in, "ln2"
                )
                mu2, rs2, nrsmu2 = _ln_stats_post(
                    nc, tiny, msel_sb, eps_t, ar2_in, ar2_out, rg_all, "ln2"
                )
                cb1 = const.tile([128, FM], F32, tag="cb1")
                nc.vector.tensor_scalar(
                    out=cb1, in0=w1sum_sb, scalar1=nrsmu2, scalar2=None,
                    op0=ALU.mult,
                )
                nc.vector.tensor_tensor(out=cb1, in0=cb1, in1=b1_sb, op=ALU.add)

                # ---- phase E: FFN (FFN1 on raw r; LN2 folded into the
                # Relu activation's scale/bias; deep PSUM pool rides out
                # the AllReduce latency) ----
                with tc.tile_pool(name="ffn", bufs=1) as pffn, \
                     tc.tile_pool(name="w1p", bufs=2) as w1p, \
                     tc.tile_pool(name="w2p", bufs=2) as w2p, \
                     tc.tile_pool(name="fout", bufs=2) as fop, \
                     tc.tile_pool(name="psf", bufs=6, space="PSUM") as psf:
                    f_T = pffn.tile([128, FM, T], BF16, tag="fT")
                    for m in range(FM):
                        w1c = w1p.tile([128, KC, 128], BF16, tag="w1c")
                        nc.sync.dma_start(
                            out=w1c,
                            in_=w1_t.ap()[:, m * 128:(m + 1) * 128]
                            .rearrange("(kc p) n -> p kc n", p=128),
                        )
                        pt = psf.tile([128, 512], F32, tag="psf")
                        for kc in range(KC):
                            nc.tensor.matmul(
                                pt, w1c[:, kc, :], rbf[:, kc, :],
                                start=(kc == 0), stop=(kc == KC - 1),
                            )
                        nc.scalar.activation(
                            out=f_T[:, m, :], in_=pt, func=AF.Relu,
                            bias=cb1[:, m:m + 1], scale=rs2,
                        )

                    for m in range(KC):
                        w2c = w2p.tile([128, FM, 128], BF16, tag="w2c")
                        nc.sync.dma_start(
                            out=w2c,
                            in_=w2_t.ap()[:, m * 128:(m + 1) * 128]
                            .rearrange("(kc p) n -> p kc n", p=128),
                        )
                        pt = psf.tile([128, 512], F32, tag="psf")
                        for kc in range(FM):
                            nc.tensor.matmul(
                                pt, w2c[:, kc, :], f_T[:, kc, :],
                                start=(kc == 0), stop=(kc == FM - 1),
                            )
                        tmp = fop.tile([128, T], F32, tag="ftmp")
                        nc.scalar.activation(
                            out=tmp, in_=pt, func=AF.Identity,
                            bias=b2_sb[:, m:m + 1],
                        )
                        fin = fop.tile([128, T], F32, tag="fin")
                        nc.vector.tensor_tensor(
                            out=fin, in0=tmp, in1=x_T[:, m, :], op=ALU.add
                        )
                        nc.sync.dma_start(
                            out=out_t.ap().rearrange(
                                "(kc p) t -> p kc t", p=128
                            )[:, m, :],
                            in_=fin,
                        )

    nc.compile()
    return nc


def _get_nc():
    if "nc" not in _CACHE:
        _CACHE["nc"] = _build()
    return _CACHE["nc"]


def _prep_in_maps(inputs):
    x = np.asarray(inputs["x"], np.float32)
    common = {}
    ws = {}
    for name in ("wq", "wk", "wv", "wo", "w1", "w2"):
        ws[name] = np.asarray(inputs[name], np.float32)
        common[name + "_t"] = np.ascontiguousarray(
            ws[name].T
        ).astype(ml_dtypes.bfloat16)
    bq = np.asarray(inputs["bq"], np.float32)
    bk = np.asarray(inputs["bk"], np.float32)
    bv = np.asarray(inputs["bv"], np.float32)
    bo = np.asarray(inputs["bo"], np.float32)
    b1 = np.asarray(inputs["b1"], np.float32)
    b2 = np.asarray(inputs["b2"], np.float32)
    common["bq_s"] = np.ascontiguousarray(bq.reshape(KC, 128).T)
    common["bk_s"] = np.ascontiguousarray(bk.reshape(KC, 128).T)
    common["bv_r"] = bv.reshape(1, D)
    common["bo_s"] = np.ascontiguousarray(bo.reshape(KC, 128).T)
    common["b1_s"] = np.ascontiguousarray(b1.reshape(FM, 128).T)
    common["b2_s"] = np.ascontiguousarray(b2.reshape(KC, 128).T)
    common["wqsum_s"] = np.ascontiguousarray(
        ws["wq"].sum(axis=1).reshape(KC, 128).T
    )
    common["wksum_s"] = np.ascontiguousarray(
        ws["wk"].sum(axis=1).reshape(KC, 128).T
    )
    common["wvsum_r"] = ws["wv"].sum(axis=1).reshape(1, D)
    common["w1sum_s"] = np.ascontiguousarray(
        ws["w1"].sum(axis=1).reshape(FM, 128).T
    )

    xf = x.reshape(B * L, D)
    in_maps = []
    for c in range(NCORES):
        m = dict(common)
        m["x_t"] = np.ascontiguousarray(xf[c * T:(c + 1) * T].T)
        m["msel"] = (
            np.array([[1, 1, 0, 0]], np.float32)
            if c // RANKS == 0
            else np.array([[0, 0, 1, 1]], np.float32)
        )
        in_maps.append(m)
    return in_maps


def _assemble(res):
    out = np.empty((B * L, D), np.float32)
    for c in range(NCORES):
        out[c * T:(c + 1) * T] = res.results[c]["out_t"].T
    return out.reshape(B, L, D)


def kernel(**inputs):
    nc = _get_nc()
    in_maps = _prep_in_maps(inputs)
    res = bass_utils.run_bass_kernel_spmd(
        nc, in_maps, core_ids=list(range(NCORES))
    )
    return _assemble(res)


def traced_run(inputs):
    nc = _get_nc()
    in_maps = _prep_in_maps(inputs)
    return bass_utils.run_bass_kernel_spmd(
        nc, in_maps, core_ids=list(range(NCORES)), trace=True
    )


# revision 19
# speedup vs baseline: 1.0974x; 1.0091x over previous
"""Trainium2 Bass kernel for nn_EncoderBlock (B=2, L=2048, D=1024, H=16, FF=4096).

Sharding: sequence-parallel over the 4096 tokens across 8 cores (512 tokens
per core; cores 0-3 own batch 0, cores 4-7 own batch 1). Per-core work is
dense (full weights). Collectives (all overlapped with compute):
  - AllGather (4-core groups) of the PRE-normalization Q projection (bf16),
    triggered first so the runtime's collective bootstrap barrier and the
    NEFF-launch skew burn during the K/V matmuls. The LN fixup is affine
    and per-partition with identical stats across the group, so it is
    applied to the gathered result instead.
  - AllReduce (8 cores) of masked LayerNorm partial sums (LN1, LN2), 16B
    each. QKV / FFN1 matmuls run on the RAW input concurrently; the
    normalization is applied afterwards as an affine fixup:
    W @ ((x - mu) * rs) = rs * (W @ x) - rs * mu * rowsum(W).
  - 8x ReduceScatter (4-core groups) of partial attention numerators and
    softmax denominators (additive across key shards); each core receives
    the full-key sums for its own 512 queries. Pipelined per 2-head group,
    with normalization interleaved one group behind the triggers.

Each core scores ALL 2048 queries of its batch against its LOCAL 512
keys/values. Activations keep the feature dim on partitions and tokens on
the free axis so matmuls contract along partitions with no transposes.
Weights are pre-transposed and cast to bf16 on the host. Softmax
denominators come from a ones-column appended to V (row 64 of the att@v
accumulator).

Dtypes: bf16 operands for all matmuls; fp32 stats/residuals/PSUM accum.
"""

import sys

sys.path.insert(0, "/opt/trn_rl_repo")

from contextlib import ExitStack  # noqa: E402

import numpy as np  # noqa: E402
import ml_dtypes  # noqa: E402

import concourse.bass as bass  # noqa: E402
import concourse.mybir as mybir  # noqa: E402
import concourse.tile as tile  # noqa: E402
from concourse import bacc, bass_utils  # noqa: E402

B, L, D, H, FF = 2, 2048, 1024, 16, 4096
DH = D // H  # 64
NCORES = 8
RANKS = 4  # cores per batch group
T = B * L // NCORES  # 512 tokens per core
KC = D // 128  # 8 contraction chunks of 128
HP = H // 2  # 8 head-pairs (2 heads per 128-partition chunk)
FM = FF // 128  # 32 ff chunks
NTOT = float(L * D)  # layernorm element count per batch
EPS = 1e-5
SCALE = 1.0 / np.sqrt(np.float32(H))  # faithful to source bug: 1/sqrt(H)

F32 = mybir.dt.float32
BF16 = mybir.dt.bfloat16

Q_ELEMS = 128 * HP * T  # qpre [128, 8, 512] bf16
HG = 2  # heads per ReduceScatter group
NRS = H // HG  # 8 ReduceScatter ops

_CACHE = {}


def _ap(t, offset, dims):
    """Manual AP over a dram tile: dims = [(step, count), ...], partition first."""
    return bass.AP(
        tensor=t.tensor, offset=t.offset + offset, ap=[[s, c] for s, c in dims]
    )


def _ln_stats_pre(nc, const, tiny, ps, src, msel_sb, ones, ar_in, pfx,
                  defer_send=False):
    """Partial LN sums of src -> masked [1,4] staged in ar_in (DRAM).

    With defer_send the staging DMA is left to the caller (returns sb4),
    so the AllReduce's readiness can be sequenced after other collectives.
    """
    AF = mybir.ActivationFunctionType
    s_part = tiny.tile([128, 1], F32, tag=pfx + "_s")
    nc.vector.tensor_reduce(
        out=s_part, in_=src, axis=mybir.AxisListType.XY, op=mybir.AluOpType.add
    )
    junk = const.tile([128, KC, T], BF16, tag="junk")
    q_part = tiny.tile([128, 1], F32, tag=pfx + "_q")
    nc.scalar.activation(out=junk, in_=src, func=AF.Square, accum_out=q_part)
    st2 = tiny.tile([128, 2], F32, tag=pfx + "_st2")
    nc.vector.tensor_copy(out=st2[:, 0:1], in_=s_part)
    nc.vector.tensor_copy(out=st2[:, 1:2], in_=q_part)
    ps_st = ps.tile([1, 2], F32, tag="ps")
    nc.tensor.matmul(ps_st, ones, st2, start=True, stop=True)
    sb4 = tiny.tile([1, 4], F32, tag=pfx + "_sb4")
    nc.scalar.copy(out=sb4[0:1, 0:2], in_=ps_st)
    nc.scalar.copy(out=sb4[0:1, 2:4], in_=ps_st)
    nc.vector.tensor_mul(out=sb4, in0=sb4, in1=msel_sb)
    if defer_send:
        return sb4
    nc.sync.dma_start(out=ar_in, in_=sb4)


def _ln_stats_post(nc, tiny, msel_sb, eps_t, ar_in, ar_out, rg_all, pfx):
    """AllReduce the staged sums; derive (mu_b, rs_b, nrsmu_b) [128,1]."""
    AF = mybir.ActivationFunctionType
    nc.gpsimd.collective_compute(
        "AllReduce", mybir.AluOpType.add, replica_groups=rg_all,
        ins=[ar_in.opt()], outs=[ar_out.opt()],
    )
    r4 = tiny.tile([1, 4], F32, tag=pfx + "_r4")
    nc.sync.dma_start(out=r4, in_=ar_out)
    nc.vector.tensor_mul(out=r4, in0=r4, in1=msel_sb)
    sq2 = tiny.tile([1, 2], F32, tag=pfx + "_sq2")
    nc.vector.tensor_tensor(
        out=sq2, in0=r4[0:1, 0:2], in1=r4[0:1, 2:4], op=mybir.AluOpType.add
    )
    mean = tiny.tile([1, 1], F32, tag=pfx + "_mean")
    nc.scalar.mul(out=mean, in_=sq2[0:1, 0:1], mul=1.0 / NTOT)
    e2 = tiny.tile([1, 1], F32, tag=pfx + "_e2")
    nc.scalar.mul(out=e2, in_=sq2[0:1, 1:2], mul=1.0 / NTOT)
    musq = tiny.tile([1, 1], F32, tag=pfx + "_musq")
    nc.vector.tensor_mul(out=musq, in0=mean, in1=mean)
    var = tiny.tile([1, 1], F32, tag=pfx + "_var")
    nc.vector.tensor_tensor(
        out=var, in0=e2, in1=musq, op=mybir.AluOpType.subtract
    )
    sd = tiny.tile([1, 1], F32, tag=pfx + "_sd")
    nc.scalar.activation(out=sd, in_=var, func=AF.Sqrt, bias=eps_t)
    rs = tiny.tile([1, 1], F32, tag=pfx + "_rs")
    nc.vector.reciprocal(out=rs, in_=sd)
    rsmu = tiny.tile([1, 1], F32, tag=pfx + "_rsmu")
    nc.vector.tensor_mul(out=rsmu, in0=mean, in1=rs)
    nrsmu = tiny.tile([1, 1], F32, tag=pfx + "_nrsmu")
    nc.scalar.mul(out=nrsmu, in_=rsmu, mul=-1.0)
    mu_b = tiny.tile([128, 1], F32, tag=pfx + "_mub")
    rs_b = tiny.tile([128, 1], F32, tag=pfx + "_rsb")
    nrsmu_b = tiny.tile([128, 1], F32, tag=pfx + "_nmb")
    nc.gpsimd.partition_broadcast(mu_b, mean)
    nc.gpsimd.partition_broadcast(rs_b, rs)
    nc.gpsimd.partition_broadcast(nrsmu_b, nrsmu)
    return mu_b, rs_b, nrsmu_b


def _build():
    nc = bacc.Bacc("TRN2", target_bir_lowering=False, debug=False,
                   num_devices=NCORES)

    x_t = nc.dram_tensor("x_t", [D, T], F32, kind="ExternalInput")
    wq_t = nc.dram_tensor("wq_t", [D, D], BF16, kind="ExternalInput")
    wk_t = nc.dram_tensor("wk_t", [D, D], BF16, kind="ExternalInput")
    wv_t = nc.dram_tensor("wv_t", [D, D], BF16, kind="ExternalInput")
    wo_t = nc.dram_tensor("wo_t", [D, D], BF16, kind="ExternalInput")
    w1_t = nc.dram_tensor("w1_t", [D, FF], BF16, kind="ExternalInput")
    w2_t = nc.dram_tensor("w2_t", [FF, D], BF16, kind="ExternalInput")
    bq_s = nc.dram_tensor("bq_s", [128, KC], F32, kind="ExternalInput")
    bk_s = nc.dram_tensor("bk_s", [128, KC], F32, kind="ExternalInput")
    bv_r = nc.dram_tensor("bv_r", [1, D], F32, kind="ExternalInput")
    bo_s = nc.dram_tensor("bo_s", [128, KC], F32, kind="ExternalInput")
    b1_s = nc.dram_tensor("b1_s", [128, FM], F32, kind="ExternalInput")
    b2_s = nc.dram_tensor("b2_s", [128, KC], F32, kind="ExternalInput")
    wqsum_s = nc.dram_tensor("wqsum_s", [128, KC], F32, kind="ExternalInput")
    wksum_s = nc.dram_tensor("wksum_s", [128, KC], F32, kind="ExternalInput")
    wvsum_r = nc.dram_tensor("wvsum_r", [1, D], F32, kind="ExternalInput")
    w1sum_s = nc.dram_tensor("w1sum_s", [128, FM], F32, kind="ExternalInput")
    msel = nc.dram_tensor("msel", [1, 4], F32, kind="ExternalInput")
    out_t = nc.dram_tensor("out_t", [D, T], F32, kind="ExternalOutput")

    rg_all = [list(range(NCORES))]
    rg_grp = [[0, 1, 2, 3], [4, 5, 6, 7]]
    AF = mybir.ActivationFunctionType
    ALU = mybir.AluOpType

    with tile.TileContext(nc) as tc, ExitStack() as ctx:
        const = ctx.enter_context(tc.tile_pool(name="const", bufs=1))
        tiny = ctx.enter_context(tc.tile_pool(name="tiny", bufs=1))
        ps = ctx.enter_context(tc.tile_pool(name="ps", bufs=2, space="PSUM"))
        dram = ctx.enter_context(tc.tile_pool(name="dram", bufs=1, space="DRAM"))

        # dram bounce buffers
        ar1_in = dram.tile([1, 4], F32, tag="ar1i")
        ar1_out = dram.tile([1, 4], F32, tag="ar1o")
        ar2_in = dram.tile([1, 4], F32, tag="ar2i")
        ar2_out = dram.tile([1, 4], F32, tag="ar2o")
        q_in = dram.tile([1, Q_ELEMS], BF16, tag="qin")
        q_out = dram.tile([1, RANKS * Q_ELEMS], BF16, tag="qout")
        rs_ins = [
            dram.tile([1, RANKS * HG * (DH + 1) * T], BF16, tag=f"rsi{g}",
                      name=f"rsi{g}")
            for g in range(NRS)
        ]
        rs_outs = [
            dram.tile([1, HG * (DH + 1) * T], BF16, tag=f"rso{g}",
                      name=f"rso{g}")
            for g in range(NRS)
        ]

        # ---- phase A: x load + LN1 partial sums ----
        x_T = const.tile([128, KC, T], F32, tag="xT")
        nc.sync.dma_start(
            out=x_T, in_=x_t.ap().rearrange("(kc p) t -> p kc t", p=128)
        )

        bq_sb = const.tile([128, KC], F32, tag="bq")
        bk_sb = const.tile([128, KC], F32, tag="bk")
        bo_sb = const.tile([128, KC], F32, tag="bo")
        b1_sb = const.tile([128, FM], F32, tag="b1")
        b2_sb = const.tile([128, KC], F32, tag="b2")
        wqsum_sb = const.tile([128, KC], F32, tag="wqs")
        wksum_sb = const.tile([128, KC], F32, tag="wks")
        w1sum_sb = const.tile([128, FM], F32, tag="w1s")
        nc.sync.dma_start(out=bq_sb, in_=bq_s.ap())
        nc.sync.dma_start(out=bk_sb, in_=bk_s.ap())
        nc.sync.dma_start(out=bo_sb, in_=bo_s.ap())
        nc.sync.dma_start(out=b1_sb, in_=b1_s.ap())
        nc.sync.dma_start(out=b2_sb, in_=b2_s.ap())
        nc.sync.dma_start(out=wqsum_sb, in_=wqsum_s.ap())
        nc.sync.dma_start(out=wksum_sb, in_=wksum_s.ap())
        nc.sync.dma_start(out=w1sum_sb, in_=w1sum_s.ap())
        msel_sb = const.tile([1, 4], F32, tag="msel")
        nc.sync.dma_start(out=msel_sb, in_=msel.ap())
        eps_t = const.tile([1, 1], F32, tag="eps")
        nc.vector.memset(eps_t, EPS)
        ones = const.tile([128, 1], F32, tag="ones")
        nc.vector.memset(ones, 1.0)

        xb = const.tile([128, KC, T], BF16, tag="xb")
        nc.vector.tensor_copy(out=xb, in_=x_T)
        ln1_sb4 = _ln_stats_pre(
            nc, const, tiny, ps, x_T, msel_sb, ones, ar1_in, "ln1",
            defer_send=True,
        )

        with tc.tile_pool(name="po", bufs=1) as po_pool, \
             tc.tile_pool(name="pd", bufs=1) as pd:
            o_T = po_pool.tile([128, KC, T], BF16, tag="oT")
            wo_sb = pd.tile([128, KC, D], BF16, tag="wo")
            nc.scalar.dma_start(
                out=wo_sb,
                in_=wo_t.ap().rearrange("(kc p) n -> p kc n", p=128),
            )

            with tc.tile_pool(name="pq", bufs=1) as pq:
                k_loc = pq.tile([128, HP, T], BF16, tag="k_loc")
                v_send = pq.tile([128, 4, H, DH + 1], BF16, tag="v_send")
                qpre = pq.tile([128, HP, T], BF16, tag="qpre")
                kpre = pq.tile([128, HP, T], BF16, tag="kpre")
                vpre = pq.tile([128, 4, H, DH], BF16, tag="vpre")

                # ---- phase B: QKV on raw x. Q first: its (pre-fixup)
                # AllGather is the first collective on the stream, so the
                # bootstrap barrier burns during the K/V matmuls. ----
                with tc.tile_pool(name="wpool", bufs=2) as wp, \
                     tc.tile_pool(name="psq", bufs=4, space="PSUM") as psq:
                    for wdram, dst in ((wq_t, qpre), (wk_t, kpre)):
                        w_sb = wp.tile([128, KC, D], BF16, tag="w")
                        nc.sync.dma_start(
                            out=w_sb,
                            in_=wdram.ap().rearrange(
                                "(kc p) n -> p kc n", p=128
                            ),
                        )
                        for hp in range(HP):
                            pt = psq.tile([128, T], F32, tag="psq")
                            for kc in range(KC):
                                nc.tensor.matmul(
                                    pt,
                                    w_sb[:, kc, hp * 128:(hp + 1) * 128],
                                    xb[:, kc, :],
                                    start=(kc == 0),
                                    stop=(kc == KC - 1),
                                )
                            nc.vector.tensor_copy(out=dst[:, hp, :], in_=pt)
                        if dst is qpre:
                            nc.scalar.dma_start(
                                out=_ap(
                                    q_in, 0, [(HP * T, 128), (T, HP), (1, T)]
                                ),
                                in_=qpre,
                            )
                            nc.gpsimd.collective_compute(
                                "AllGather", mybir.AluOpType.bypass,
                                replica_groups=rg_grp,
                                ins=[q_in.opt()], outs=[q_out.opt()],
                            )
                            # stage the LN1 sums only now (scalar queue,
                            # behind the q pack) so the AllReduce cannot
                            # jump ahead of the AllGather on the CC stream
                            nc.scalar.dma_start(out=ar1_in, in_=ln1_sb4)
                            # LN1 AllReduce queues on the stream behind
                            # the AllGather; result needed only for fixups
                            mu1, rs1, nrsmu1 = _ln_stats_post(
                                nc, tiny, msel_sb, eps_t, ar1_in, ar1_out,
                                rg_all, "ln1",
                            )
                            cbq = const.tile([128, KC], F32, tag="cbq")
                            nc.vector.tensor_scalar(
                                out=cbq, in0=wqsum_sb, scalar1=nrsmu1,
                                scalar2=None, op0=ALU.mult,
                            )
                            nc.vector.tensor_tensor(
                                out=cbq, in0=cbq, in1=bq_sb, op=ALU.add
                            )
                            cbk = const.tile([128, KC], F32, tag="cbk")
                            nc.vector.tensor_scalar(
                                out=cbk, in0=wksum_sb, scalar1=nrsmu1,
                                scalar2=None, op0=ALU.mult,
                            )
                            nc.vector.tensor_tensor(
                                out=cbk, in0=cbk, in1=bk_sb, op=ALU.add
                            )

                    wv_sb = wp.tile([128, KC, D], BF16, tag="w")
                    nc.sync.dma_start(
                        out=wv_sb,
                        in_=wv_t.ap().rearrange("(kc p) n -> p kc n", p=128),
                    )
                    # per-column V fixup constants (cvb = bv - rs*mu*wvsum)
                    bv_bc = wp.tile([128, D], F32, tag="bv", bufs=1)
                    nc.sync.dma_start(
                        out=bv_bc, in_=bv_r.ap().to_broadcast((128, D))
                    )
                    wvsum_bc = wp.tile([128, D], F32, tag="wvs", bufs=1)
                    nc.sync.dma_start(
                        out=wvsum_bc, in_=wvsum_r.ap().to_broadcast((128, D))
                    )
                    cvb = wp.tile([128, D], F32, tag="cvb", bufs=1)
                    nc.vector.tensor_scalar(
                        out=cvb, in0=wvsum_bc, scalar1=nrsmu1, scalar2=None,
                        op0=ALU.mult,
                    )
                    nc.vector.tensor_tensor(
                        out=cvb, in0=cvb, in1=bv_bc, op=ALU.add
                    )
                    for tcnk in range(4):
                        for n in range(2):
                            pt = psq.tile([128, 512], F32, tag="psq")
                            for kc in range(KC):
                                nc.tensor.matmul(
                                    pt,
                                    xb[:, kc, tcnk * 128:(tcnk + 1) * 128],
                                    wv_sb[:, kc, n * 512:(n + 1) * 512],
                                    start=(kc == 0),
                                    stop=(kc == KC - 1),
                                )
                            nc.vector.tensor_copy(
                                out=vpre[:, tcnk, n * 8:(n + 1) * 8, :],
                                in_=pt.rearrange("p (h d) -> p h d", d=DH),
                            )
                    # k / v fixups
                    for hp in range(HP):
                        nc.vector.tensor_scalar(
                            out=k_loc[:, hp, :], in0=kpre[:, hp, :],
                            scalar1=rs1, scalar2=cbk[:, hp:hp + 1],
                            op0=ALU.mult, op1=ALU.add,
                        )
                    for tcnk in range(4):
                        vtmp = wp.tile([128, H, DH], BF16, tag="vtmp")
                        nc.vector.tensor_scalar(
                            out=vtmp, in0=vpre[:, tcnk, :, :],
                            scalar1=rs1, scalar2=None, op0=ALU.mult,
                        )
                        nc.vector.tensor_tensor(
                            out=v_send[:, tcnk, :, 0:DH],
                            in0=vtmp,
                            in1=cvb.rearrange("p (h d) -> p h d", d=DH),
                            op=ALU.add,
                        )
                    nc.vector.memset(v_send[:, :, :, DH:DH + 1], 1.0)

                # ---- phase C: attention (all queries x local keys) ----
                with tc.tile_pool(name="att", bufs=1) as patt, \
                     tc.tile_pool(name="etp", bufs=3) as etp, \
                     tc.tile_pool(name="nmp", bufs=3) as nmp, \
                     tc.tile_pool(name="ps_s", bufs=2, space="PSUM") as ps_s, \
                     tc.tile_pool(name="ps_o", bufs=2, space="PSUM") as ps_o:
                    qf = []
                    for r in range(RANKS):
                        qa = patt.tile([128, HP, T], BF16, tag=f"qa{r}",
                                       name=f"qa{r}")
                        nc.sync.dma_start(
                            out=qa,
                            in_=_ap(
                                q_out, r * Q_ELEMS,
                                [(HP * T, 128), (T, HP), (1, T)],
                            ),
                        )
                        # LN1 fixup in place on the gathered pre-Q: same
                        # batch stats on all ranks; cbq varies per partition
                        # and per hp chunk
                        for hp in range(HP):
                            nc.vector.tensor_scalar(
                                out=qa[:, hp, :], in0=qa[:, hp, :],
                                scalar1=rs1, scalar2=cbq[:, hp:hp + 1],
                                op0=ALU.mult, op1=ALU.add,
                            )
                        qf.append(qa)

                    def normalize_group(g):
                        na = nmp.tile([DH + 1, HG, T], BF16, tag="na",
                                      name="na")
                        nc.sync.dma_start(
                            out=na,
                            in_=_ap(
                                rs_outs[g], 0,
                                [(T, DH + 1), ((DH + 1) * T, HG), (1, T)],
                            ),
                        )
                        for hh in range(HG):
                            h = g * HG + hh
                            den = nmp.tile([1, T], BF16, tag="den",
                                           name="den")
                            nc.gpsimd.dma_start(
                                out=den, in_=na[DH:DH + 1, hh, :]
                            )
                            rec = nmp.tile([1, T], F32, tag="rec", name="rec")
                            nc.vector.reciprocal(out=rec, in_=den)
                            rb = nmp.tile([DH, T], F32, tag="rb", name="rb")
                            nc.gpsimd.partition_broadcast(rb, rec, channels=DH)
                            stg = nmp.tile([DH, T], BF16, tag="stg",
                                           name="stg")
                            nc.vector.tensor_tensor(
                                out=stg, in0=na[0:DH, hh, :], in1=rb,
                                op=ALU.mult,
                            )
                            nc.sync.dma_start(
                                out=o_T[(h % 2) * DH:(h % 2 + 1) * DH,
                                        h // 2, :],
                                in_=stg,
                            )

                    for h in range(H):
                        hp, off = h // 2, (h % 2) * DH
                        for r in range(RANKS):
                            po = ps_o.tile([DH + 1, T], F32, tag="ps_o")
                            for kk in (0, 2):
                                pss = ps_s.tile([128, 2, T], F32, tag="ps_s")
                                for j in range(2):
                                    kc = kk + j
                                    nc.tensor.matmul(
                                        pss[:, j, :],
                                        k_loc[off:off + DH, hp,
                                              kc * 128:(kc + 1) * 128],
                                        qf[r][off:off + DH, hp, :],
                                        start=True,
                                        stop=True,
                                    )
                                et = etp.tile([128, 2, T], BF16, tag="et")
                                nc.scalar.activation(
                                    out=et, in_=pss, func=AF.Exp,
                                    scale=float(SCALE),
                                )
                                for j in range(2):
                                    kc = kk + j
                                    nc.tensor.matmul(
                                        po,
                                        v_send[:, kc, h, :],
                                        et[:, j, :],
                                        start=(kc == 0),
                                        stop=(kc == 3),
                                    )
                            num = nmp.tile([DH + 1, T], BF16, tag="num")
                            nc.vector.tensor_copy(out=num, in_=po)
                            g = h // HG
                            nc.sync.dma_start(
                                out=_ap(
                                    rs_ins[g],
                                    (r * HG + (h % HG)) * (DH + 1) * T,
                                    [(T, DH + 1), (1, T)],
                                ),
                                in_=num,
                            )
                        if h % HG == HG - 1:
                            g = h // HG
                            nc.gpsimd.collective_compute(
                                "ReduceScatter", mybir.AluOpType.add,
                                replica_groups=rg_grp,
                                ins=[rs_ins[g].opt()],
                                outs=[rs_outs[g].opt()],
                            )
                            if g >= 2:
                                normalize_group(g - 2)
                    normalize_group(NRS - 2)
                    normalize_group(NRS - 1)

            # ---- phase D: o_proj + residual + LN2 ----
            with tc.tile_pool(name="pdt", bufs=2) as pdt:
                r_T = pd.tile([128, KC, T], F32, tag="rT")
                rbf = pd.tile([128, KC, T], BF16, tag="rbf")
                for m in range(KC):
                    pt = ps.tile([128, 512], F32, tag="ps")
                    for kc in range(KC):
                        nc.tensor.matmul(
                            pt,
                            wo_sb[:, kc, m * 128:(m + 1) * 128],
                            o_T[:, kc, :],
                            start=(kc == 0),
                            stop=(kc == KC - 1),
                        )
                    tmp = pdt.tile([128, T], F32, tag="otmp")
                    nc.scalar.activation(
                        out=tmp, in_=pt, func=AF.Identity,
                        bias=bo_sb[:, m:m + 1],
                    )
                    nc.vector.tensor_tensor(
                        out=r_T[:, m, :], in0=tmp, in1=x_T[:, m, :],
                        op=ALU.add,
                    )
                    nc.vector.tensor_copy(out=rbf[:, m, :], in_=r_T[:, m, :])
                _ln_stats_pre(
                    nc, const, tiny, ps, r_T, msel_sb, ones, ar2_in, "ln2"
                )
                mu2, rs2, nrsmu2 = _ln_stats_post(
                    nc, tiny, msel_sb, eps_t, ar2_in, ar2_out, rg_all, "ln2"
                )
                cb1 = const.tile([128, FM], F32, tag="cb1")
                nc.vector.tensor_scalar(
                    out=cb1, in0=w1sum_sb, scalar1=nrsmu2, scalar2=None,
                    op0=ALU.mult,
                )
                nc.vector.tensor_tensor(out=cb1, in0=cb1, in1=b1_sb, op=ALU.add)

                # ---- phase E: FFN (FFN1 on raw r; LN2 folded into the
                # Relu activation's scale/bias; deep PSUM pool rides out
                # the AllReduce latency) ----
                with tc.tile_pool(name="ffn", bufs=1) as pffn, \
                     tc.tile_pool(name="w1p", bufs=2) as w1p, \
                     tc.tile_pool(name="w2p", bufs=2) as w2p, \
                     tc.tile_pool(name="fout", bufs=2) as fop, \
                     tc.tile_pool(name="psf", bufs=6, space="PSUM") as psf:
                    f_T = pffn.tile([128, FM, T], BF16, tag="fT")
                    for m in range(FM):
                        w1c = w1p.tile([128, KC, 128], BF16, tag="w1c")
                        nc.sync.dma_start(
                            out=w1c,
                            in_=w1_t.ap()[:, m * 128:(m + 1) * 128]
                            .rearrange("(kc p) n -> p kc n", p=128),
                        )
                        pt = psf.tile([128, 512], F32, tag="psf")
                        for kc in range(KC):
                            nc.tensor.matmul(
                                pt, w1c[:, kc, :], rbf[:, kc, :],
                                start=(kc == 0), stop=(kc == KC - 1),
                            )
                        nc.scalar.activation(
                            out=f_T[:, m, :], in_=pt, func=AF.Relu,
                            bias=cb1[:, m:m + 1], scale=rs2,
                        )

                    for m in range(KC):
                        w2c = w2p.tile([128, FM, 128], BF16, tag="w2c")
                        nc.sync.dma_start(
                            out=w2c,
                            in_=w2_t.ap()[:, m * 128:(m + 1) * 128]
                            .rearrange("(kc p) n -> p kc n", p=128),
                        )
                        pt = psf.tile([128, 512], F32, tag="psf")
                        for kc in range(FM):
                            nc.tensor.matmul(
                                pt, w2c[:, kc, :], f_T[:, kc, :],
                                start=(kc == 0), stop=(kc == FM - 1),
                            )
                        tmp = fop.tile([128, T], F32, tag="ftmp")
                        nc.scalar.activation(
                            out=tmp, in_=pt, func=AF.Identity,
                            bias=b2_sb[:, m:m + 1],
                        )
                        fin = fop.tile([128, T], F32, tag="fin")
                        nc.vector.tensor_tensor(
                            out=fin, in0=tmp, in1=x_T[:, m, :], op=ALU.add
                        )
                        nc.sync.dma_start(
                            out=out_t.ap().rearrange(
                                "(kc p) t -> p kc t", p=128
                            )[:, m, :],
                            in_=fin,
                        )

    nc.compile()
    return nc


def _get_nc():
    if "nc" not in _CACHE:
        _CACHE["nc"] = _build()
    return _CACHE["nc"]


def _prep_in_maps(inputs):
    x = np.asarray(inputs["x"], np.float32)
    common = {}
    ws = {}
    for name in ("wq", "wk", "wv", "wo", "w1", "w2"):
        ws[name] = np.asarray(inputs[name], np.float32)
        common[name + "_t"] = np.ascontiguousarray(
            ws[name].T
        ).astype(ml_dtypes.bfloat16)
    bq = np.asarray(inputs["bq"], np.float32)
    bk = np.asarray(inputs["bk"], np.float32)
    bv = np.asarray(inputs["bv"], np.float32)
    bo = np.asarray(inputs["bo"], np.float32)
    b1 = np.asarray(inputs["b1"], np.float32)
    b2 = np.asarray(inputs["b2"], np.float32)
    common["bq_s"] = np.ascontiguousarray(bq.reshape(KC, 128).T)
    common["bk_s"] = np.ascontiguousarray(bk.reshape(KC, 128).T)
    common["bv_r"] = bv.reshape(1, D)
    common["bo_s"] = np.ascontiguousarray(bo.reshape(KC, 128).T)
    common["b1_s"] = np.ascontiguousarray(b1.reshape(FM, 128).T)
    common["b2_s"] = np.ascontiguousarray(b2.reshape(KC, 128).T)
    common["wqsum_s"] = np.ascontiguousarray(
        ws["wq"].sum(axis=1).reshape(KC, 128).T
    )
    common["wksum_s"] = np.ascontiguousarray(
        ws["wk"].sum(axis=1).reshape(KC, 128).T
    )
    common["wvsum_r"] = ws["wv"].sum(axis=1).reshape(1, D)
    common["w1sum_s"] = np.ascontiguousarray(
        ws["w1"].sum(axis=1).reshape(FM, 128).T
    )

    xf = x.reshape(B * L, D)
    in_maps = []
    for c in range(NCORES):
        m = dict(common)
        m["x_t"] = np.ascontiguousarray(xf[c * T:(c + 1) * T].T)
        m["msel"] = (
            np.array([[1, 1, 0, 0]], np.float32)
            if c // RANKS == 0
            else np.array([[0, 0, 1, 1]], np.float32)
        )
        in_maps.append(m)
    return in_maps


def _assemble(res):
    out = np.empty((B * L, D), np.float32)
    for c in range(NCORES):
        out[c * T:(c + 1) * T] = res.results[c]["out_t"].T
    return out.reshape(B, L, D)


def kernel(**inputs):
    nc = _get_nc()
    in_maps = _prep_in_maps(inputs)
    res = bass_utils.run_bass_kernel_spmd(
        nc, in_maps, core_ids=list(range(NCORES))
    )
    return _assemble(res)


def traced_run(inputs):
    nc = _get_nc()
    in_maps = _prep_in_maps(inputs)
    return bass_utils.run_bass_kernel_spmd(
        nc, in_maps, core_ids=list(range(NCORES)), trace=True
    )
